# revision 17
# baseline (speedup 1.0000x reference)
"""CGCNN message-passing kernel for 8 Trainium2 NeuronCores.

Strategy:
  - Nodes partitioned contiguously across 8 cores (NPC nodes each, padded).
  - Edges sharded by dst owner, sorted by dst, padded per 128-node block to
    multiples of 128 ("groups"); per-block group counts equalized across
    cores so a single SPMD program fits all cores.
  - Per layer: each core computes hs = h@wsrc and hd = h@wdst + conv_b for
    its own nodes; hs tables are AllGathered (bf16) into a global row table;
    per-edge rows fetched with indirect-DMA gathers; ze = ef@we via K=20
    matmuls; per-edge sum injected into PSUM via identity matmul.
  - msg = sigmoid(zg)*softplus(zf) computed with only the natural_log_exp
    ACT table set: msg = (1/(1+exp(zg)) - 1) * ln(1+exp(zf)) and the sign
    is folded into the one-hot S matrices (entries are -1).
  - Aggregation: agg^T[feat, nodes] += msg_j^T @ S_j accumulated in PSUM per
    128-node block; BN stats via free-dim reduction + tiny AllReduce;
    rsqrt = exp(-0.5*ln(var+eps)). Mean-pool via one-hot matmul + AllReduce;
    small FC head replicated on every core.

kernel(**inputs) -> np.ndarray (B, 1) float32. Self-contained (shapes
hardcoded below; no file reads).
"""
import sys

sys.path.insert(0, "/opt/trn_rl_repo")

import numpy as np
from ml_dtypes import bfloat16

import concourse.bacc as bacc
import concourse.bass as bass
import concourse.mybir as mybir
import concourse.tile as tile
from concourse.bass_utils import run_bass_kernel_spmd
from concourse.masks import make_identity

# Exp lives in act-table set "exp_and_others" and Ln in "natural_log"; the
# table-load inserter picks the first set containing each function, so a
# kernel alternating Exp/Ln reloads ACT tables on every use (~1.3us each,
# thousands of times).  Strip those entries so both resolve to the combined
# "natural_log_exp_and_others" set and the load hoists out of all loops.
_orig_get_act_tables = bacc.get_activation_tables


def _patched_get_act_tables(arch):
    t = _orig_get_act_tables(arch)
    AF = mybir.ActivationFunctionType
    for name, fs in t.items():
        if name == "exp_and_others":
            fs.discard(AF.Exp)
        elif name == "natural_log":
            fs.discard(AF.Ln)
    return t


bacc.get_activation_tables = _patched_get_act_tables

F32 = mybir.dt.float32
BF16 = mybir.dt.bfloat16
I32 = mybir.dt.int32
AF = mybir.ActivationFunctionType
ALU = mybir.AluOpType

NCORES = 8
GROUP = 128
SUPER = 6            # groups per supertile: z-PSUM = 3 banks, double buffered
EPS = 1e-5


class Cfg:
    def __init__(self, N, E, B, ND, ED, H, NL):
        self.N, self.E, self.B = N, E, B
        self.ND, self.ED, self.H, self.NL = ND, ED, H, NL
        per = (N + NCORES - 1) // NCORES
        self.NPC = (per + 127) // 128 * 128
        self.NB = self.NPC // 128
        self.NPAD = self.NPC * NCORES


DEFAULT_CFG = Cfg(N=100000, E=1600000, B=128, ND=91, ED=20, H=128, NL=3)


# --------------------------------------------------------------------------
# Host-side preprocessing
# --------------------------------------------------------------------------
def _preprocess(cfg, inputs):
    N, B, H, ED, ND, NL = cfg.N, cfg.B, cfg.H, cfg.ED, cfg.ND, cfg.NL
    NPC, NB = cfg.NPC, cfg.NB

    src = np.asarray(inputs["edge_index"][0], np.int64)
    dst = np.asarray(inputs["edge_index"][1], np.int64)
    ef32 = np.asarray(inputs["edge_feats"], np.float32)
    batch_np = np.asarray(inputs["batch"], np.int64)

    order = np.argsort(dst, kind="stable")
    src_s, dst_s, e_s = src[order], dst[order], order

    BK = 32768                       # int16 gather bucket width
    NBK = -(-(NPC * NCORES) // BK)   # buckets over padded node space

    starts = np.searchsorted(dst_s, np.arange(NCORES) * NPC)
    ends = np.searchsorted(dst_s, np.arange(NCORES) * NPC + NPC)

    # per-core, per-block, per-bucket edge counts
    cnt = np.zeros((NCORES, NB, NBK), np.int64)
    core_block = []
    for c in range(NCORES):
        lo, hi = int(starts[c]), int(ends[c])
        cdstl = dst_s[lo:hi] - c * NPC
        cblk = cdstl // 128
        bstart = np.searchsorted(cblk, np.arange(NB))
        bend = np.searchsorted(cblk, np.arange(NB) + 1)
        core_block.append((lo, bstart, bend))
        for b in range(NB):
            bk = src_s[lo + bstart[b]:lo + bend[b]] // BK
            cnt[c, b] = np.bincount(bk, minlength=NBK)

    # shared quotas: per block, per bucket, max over cores, rounded to x128
    quota = -(-cnt.max(axis=0) // GROUP) * GROUP          # [NB, NBK]
    quota = np.maximum(quota, 0)
    gpb = quota.sum(axis=1) // GROUP                      # groups per block
    gpb = np.maximum(gpb, 1)
    # keep gpb consistent with quota (empty blocks get one dummy group in bucket 0)
    for b in range(NB):
        if quota[b].sum() == 0:
            quota[b, 0] = GROUP
    gpb = quota.sum(axis=1) // GROUP
    G = int(gpb.sum())

    grp_block = np.repeat(np.arange(NB), gpb)
    last_grp_of_block = (np.cumsum(gpb) - 1)

    conv_b = np.asarray(inputs["conv_b"], np.float32)
    counts = np.maximum(np.bincount(batch_np, minlength=B), 1.0)
    counts_inv = (1.0 / counts).astype(np.float32).reshape(B, 1)
    wsd = [np.concatenate([np.asarray(inputs["conv_wsrc"][i], np.float32),
                           np.asarray(inputs["conv_wdst"][i], np.float32)],
                          axis=1) for i in range(NL)]             # [H, 4H]
    nf = np.asarray(inputs["node_feats"], np.float32)

    def wrap16(flat):
        # dma_gather idx layout: [128, n//16] wrapped in 16 partitions, x8
        n = flat.shape[0]
        w = flat.reshape(n // 16, 16).T
        return np.tile(w, (8, 1)).astype(np.int16)

    hs_cols = int(quota.sum()) // 16                      # total idx cols
    in_maps = []
    for c in range(NCORES):
        lo, bstart, bend = core_block[c]
        src_c = src_s
        dst_c = dst_s
        S = np.zeros((128, G * 128), np.float32)
        D = np.zeros((128, G * 128), np.float32)
        efT = np.zeros((ED, G * 128), np.float32)
        hs_idx = np.zeros((128, hs_cols), np.int16)
        hcol = 0
        g0 = 0
        for b in range(NB):
            eb = slice(lo + bstart[b], lo + bend[b])
            eb_src = src_c[eb]
            eb_dstl = dst_c[eb] - c * NPC
            eb_e = e_s[eb]
            obk = np.argsort(eb_src // BK, kind="stable")
            eb_src, eb_dstl, eb_e = eb_src[obk], eb_dstl[obk], eb_e[obk]
            bk = eb_src // BK
            bko = np.searchsorted(bk, np.arange(NBK))
            bke = np.searchsorted(bk, np.arange(NBK) + 1)
            nslots = int(quota[b].sum())
            # build per-block slot arrays (flat order: slot i -> p=i%128, g=i//128)
            bsrc = np.zeros(nslots, np.int64)
            bdst = np.zeros(nslots, np.int64)
            bein = np.full(nslots, -1, np.int64)
            off = 0
            for k in range(NBK):
                q = int(quota[b, k])
                n_e = int(bke[k] - bko[k])
                assert n_e <= q, (c, b, k, n_e, q)
                if q == 0:
                    continue
                bsrc[off:off + n_e] = eb_src[bko[k]:bke[k]]
                bsrc[off + n_e:off + q] = k * BK        # pad: bucket base row
                bdst[off:off + n_e] = eb_dstl[bko[k]:bke[k]]
                bdst[off + n_e:off + q] = b * 128      # pad -> slot 0 w/ S=0
                bein[off:off + n_e] = eb_e[bko[k]:bke[k]]
                # idx for this bucket call (relative to bucket base)
                hs_idx[:, hcol:hcol + q // 16] = wrap16(
                    (bsrc[off:off + q] - k * BK).astype(np.int16))
                hcol += q // 16
                off += q
            # S (agg one-hot, [edge-slot p, node slot], entries -1) and
            # D (hd-select one-hot, [node slot, edge-slot p], entries +1)
            ii = np.arange(nslots)
            p = ii % 128
            gg = g0 + ii // 128
            real = bein >= 0
            slot = (bdst - b * 128).astype(np.int64)
            S[p[real], gg[real] * 128 + slot[real]] = -1.0
            D[slot[real], (gg * 128 + p)[real]] = 1.0
            efT[:, (gg * 128 + p)[real]] = ef32[bein[real]].T
            g0 += nslots // 128

        Bt = np.zeros((128, NB * B), np.float32)
        for t in range(NB):
            ids = np.arange(c * NPC + t * 128, c * NPC + t * 128 + 128)
            valid = ids < N
            gv = np.where(valid, batch_np[np.minimum(ids, N - 1)], 0)
            Bt[np.arange(128)[valid], t * B + gv[valid]] = 1.0

        nfT = np.zeros((ND, NPC), np.float32)
        n_real = min(NPC, max(0, N - c * NPC))
        if n_real > 0:
            nfT[:, :n_real] = nf[c * NPC:c * NPC + n_real].T

        m = {
            "nfT": nfT,
            "hs_idx": hs_idx,
            "S": S.astype(bfloat16),
            "D": D.astype(bfloat16),
            "efT": efT.astype(bfloat16),
            "Btiles": Bt.astype(bfloat16),
            "embed_w": np.asarray(inputs["embed_w"], np.float32),
            "embed_b": np.asarray(inputs["embed_b"], np.float32).reshape(H, 1),
            "counts_inv": counts_inv,
            "fc1_w": np.asarray(inputs["fc1_w"], np.float32),
            "fc1_b": np.asarray(inputs["fc1_b"], np.float32).reshape(H, 1),
            "fc_g": np.asarray(inputs["fc_bn_gamma"], np.float32).reshape(H, 1),
            "fc_be": np.asarray(inputs["fc_bn_beta"], np.float32).reshape(H, 1),
            "out_w": np.asarray(inputs["out_w"], np.float32).reshape(H, 1),
        }
        for i in range(NL):
            m[f"wsd{i}"] = wsd[i]
            m[f"we{i}"] = np.asarray(inputs["conv_we"][i], np.float32)
            m[f"biasb{i}"] = np.tile(conv_b[i][None, :], (128, 1))
            m[f"bn_g{i}"] = np.asarray(inputs["bn_gamma"][i], np.float32).reshape(H, 1)
            m[f"bn_b{i}"] = np.asarray(inputs["bn_beta"][i], np.float32).reshape(H, 1)
        in_maps.append(m)

    meta = dict(G=G, BK=BK, NBK=NBK,
                quota=quota.tolist(), gpb=gpb.tolist(),
                grp_block=grp_block.tolist(),
                last_grp_of_block=set(int(x) for x in last_grp_of_block),
                out_b=float(np.asarray(inputs["out_b"]).reshape(-1)[0]))
    return meta, in_maps


# --------------------------------------------------------------------------
# Device program
# --------------------------------------------------------------------------
def _build(cfg, meta, debug=False):
    N, B, H, ED, ND, NL = cfg.N, cfg.B, cfg.H, cfg.ED, cfg.ND, cfg.NL
    NPC, NB, NPAD = cfg.NPC, cfg.NB, cfg.NPAD
    G = meta["G"]
    BK, NBK = meta["BK"], meta["NBK"]
    quota = meta["quota"]
    gpb = meta["gpb"]
    grp_block = meta["grp_block"]
    last_grps = meta["last_grp_of_block"]
    hs_cols = sum(sum(q) for q in quota) // 16
    MAXGPB = max(gpb)
    RG = [list(range(NCORES))]

    nc = bacc.Bacc("TRN2", target_bir_lowering=False, debug=False,
                   num_devices=NCORES)

    def inp(name, shape, dt=F32):
        return nc.dram_tensor(name, shape, dt, kind="ExternalInput")

    nfT_d = inp("nfT", [ND, NPC])
    hsix_d = inp("hs_idx", [128, hs_cols], mybir.dt.int16)
    S_d = inp("S", [128, G * 128], BF16)
    D_d = inp("D", [128, G * 128], BF16)
    efT_d = inp("efT", [ED, G * 128], BF16)
    Bt_d = inp("Btiles", [128, NB * B], BF16)
    ew_d = inp("embed_w", [ND, H])
    eb_d = inp("embed_b", [H, 1])
    cinv_d = inp("counts_inv", [B, 1])
    fc1w_d = inp("fc1_w", [H, H])
    fc1b_d = inp("fc1_b", [H, 1])
    fcg_d = inp("fc_g", [H, 1])
    fcb_d = inp("fc_be", [H, 1])
    outw_d = inp("out_w", [H, 1])
    wsd_d = [inp(f"wsd{i}", [H, 4 * H]) for i in range(NL)]
    we_d = [inp(f"we{i}", [ED, 2 * H]) for i in range(NL)]
    bb_d = [inp(f"biasb{i}", [128, 2 * H]) for i in range(NL)]
    bng_d = [inp(f"bn_g{i}", [H, 1]) for i in range(NL)]
    bnb_d = [inp(f"bn_b{i}", [H, 1]) for i in range(NL)]
    out_d = nc.dram_tensor("out", [B, 1], F32, kind="ExternalOutput")
    dbg = {}
    if debug:
        for nm, shp, dt in [("dbg_h0", [128, NPC], BF16),
                            ("dbg_agg0", [128, NPC], F32),
                            ("dbg_st0", [H, 2], F32),
                            ("dbg_h1", [128, NPC], BF16),
                            ("dbg_h3", [128, NPC], BF16),
                            ("dbg_g", [B, H], F32),
                            ("dbg_hh", [128, SUPER * 256], BF16),
                            ("dbg_hsg", [128, SUPER * 256], BF16),
                            ("dbg_hdg", [128, SUPER * 256], BF16),
                            ("dbg_z", [128, SUPER * 256], F32),
                            ("dbg_msg", [128, SUPER * 128], BF16)]:
            dbg[nm] = nc.dram_tensor(nm, shp, dt, kind="ExternalOutput")

    with tile.TileContext(nc) as tc:
        with (
            tc.tile_pool(name="const", bufs=1) as cp,
            tc.tile_pool(name="dram", bufs=1, space="DRAM") as dr,
        ):
            hT_dram = dr.tile([128, NPC], BF16, tag="hT")
            agg_dram = dr.tile([128, NPC], F32, tag="agg")
            hsag_in_l, hs_full_l, stat_in_l, stat_out_l = [], [], [], []
            for i in range(NL):
                hsag_t = dr.tile([NPC, 2 * H], BF16, tag=f"hsin{i}", name=f"hsin{i}")
                hsag_in_l.append(hsag_t)
                hsf_t = dr.tile([NPAD, 2 * H], BF16, tag=f"hsfull{i}",
                                addr_space="Shared", name=f"hsfull{i}")
                hs_full_l.append(hsf_t)
                sti_t = dr.tile([H, 2], F32, tag=f"stin{i}", name=f"stin{i}")
                stat_in_l.append(sti_t)
                sto_t = dr.tile([H, 2], F32, tag=f"stout{i}", name=f"stout{i}")
                stat_out_l.append(sto_t)
            g_in = dr.tile([B, H], F32, tag="gin")
            g_out = dr.tile([B, H], F32, tag="gout")

            # ---- resident constants ----
            ew_sb = cp.tile([128, H], BF16, tag="ew")
            nc.gpsimd.dma_start(ew_sb[:ND, :], ew_d[:, :])
            eb_sb = cp.tile([H, 1], F32, tag="eb")
            nc.sync.dma_start(eb_sb[:, :], eb_d[:, :])
            wsd_sb, we_sb, bb_sb, bng_sb, bnb_sb = [], [], [], [], []
            for i in range(NL):
                w = cp.tile([H, 4 * H], BF16, tag=f"wsd{i}")
                nc.gpsimd.dma_start(w[:, :], wsd_d[i][:, :])
                wsd_sb.append(w)
                w = cp.tile([128, 2 * H], BF16, tag=f"we{i}")
                nc.gpsimd.dma_start(w[:ED, :], we_d[i][:, :])
                we_sb.append(w)
                w = cp.tile([128, 2 * H], F32, tag=f"bb{i}")
                nc.sync.dma_start(w[:, :], bb_d[i][:, :])
                bb_sb.append(w)
                w = cp.tile([H, 1], F32, tag=f"bng{i}")
                nc.sync.dma_start(w[:, :], bng_d[i][:, :])
                bng_sb.append(w)
                w = cp.tile([H, 1], F32, tag=f"bnb{i}")
                nc.sync.dma_start(w[:, :], bnb_d[i][:, :])
                bnb_sb.append(w)
            cinv_sb = cp.tile([B, 1], F32, tag="cinv")
            nc.sync.dma_start(cinv_sb[:, :], cinv_d[:, :])
            fc1w_sb = cp.tile([H, H], F32, tag="fc1w")
            nc.sync.dma_start(fc1w_sb[:, :], fc1w_d[:, :])
            fc1b_sb = cp.tile([H, 1], F32, tag="fc1b")
            nc.sync.dma_start(fc1b_sb[:, :], fc1b_d[:, :])
            fcg_sb = cp.tile([H, 1], F32, tag="fcg")
            nc.sync.dma_start(fcg_sb[:, :], fcg_d[:, :])
            fcb_sb = cp.tile([H, 1], F32, tag="fcb")
            nc.sync.dma_start(fcb_sb[:, :], fcb_d[:, :])
            outw_sb = cp.tile([H, 1], F32, tag="outw")
            nc.sync.dma_start(outw_sb[:, :], outw_d[:, :])
            id_bf = cp.tile([128, 128], BF16, tag="idbf")
            make_identity(nc, id_bf[:])
            id_f32 = cp.tile([128, 128], F32, tag="idf32")
            make_identity(nc, id_f32[:])
            scsh_sb = cp.tile([H, 2], F32, tag="scsh")  # per-layer bn scale/shift
            # gather index table resident in SBUF (reused every layer)
            hsix_sb = cp.tile([128, hs_cols], mybir.dt.int16, tag="hsix")
            nc.sync.dma_start(hsix_sb[:, :], hsix_d[:, :])
            # hd (dst-half of conv + bias) resident: [node-in-block, NB*2H]
            hd_sb = cp.tile([128, NB * 2 * H], BF16, tag="hdsb")

            # ============ embed ============
            with (
                tc.tile_pool(name="emb", bufs=3) as ep,
                tc.tile_pool(name="embp", bufs=2, space="PSUM") as epp,
            ):
                for t in range(NB):
                    nftb = ep.tile([128, 128], BF16, tag="nftb")
                    nc.gpsimd.dma_start(nftb[:ND, :],
                                        nfT_d[:, t * 128:(t + 1) * 128])
                    ps = epp.tile([128, 128], F32, space="PSUM", tag="embp")
                    nc.tensor.matmul(ps[:], lhsT=ew_sb[:ND, :],
                                     rhs=nftb[:ND, :], start=True, stop=True)
                    h0 = ep.tile([128, 128], BF16, tag="h0")
                    nc.vector.scalar_tensor_tensor(
                        out=h0[:], in0=ps[:], scalar=1.0,
                        in1=eb_sb[:].to_broadcast([128, 128]),
                        op0=ALU.mult, op1=ALU.add)
                    nc.sync.dma_start(hT_dram[:, t * 128:(t + 1) * 128], h0[:])

            if debug:
                nc.sync.dma_start(dbg["dbg_h0"][:, :], hT_dram[:, :])
            # ============ layers ============
            g_pool_psum = None
            for li in range(NL):
                hsag_in = hsag_in_l[li]
                hs_full = hs_full_l[li]
                stat_in = stat_in_l[li]
                stat_out = stat_out_l[li]
                # ---- phase H ----
                with (
                    tc.tile_pool(name="ph", bufs=4) as hp,
                    tc.tile_pool(name="php", bufs=2, space="PSUM") as hpp,
                ):
                    for t in range(NB):
                        ht = hp.tile([128, 128], BF16, tag="ht")
                        nc.sync.dma_start(ht[:],
                                          hT_dram[:, t * 128:(t + 1) * 128])
                        ps = hpp.tile([128, 4 * H], F32, space="PSUM", tag="hsd")
                        nc.tensor.matmul(ps[:], lhsT=ht[:], rhs=wsd_sb[li][:],
                                         start=True, stop=True)
                        hs_row = hp.tile([128, 2 * H], BF16, tag="hsr")
                        nc.scalar.activation(hs_row[:], ps[:, :2 * H], AF.Copy)
                        nc.vector.tensor_tensor(
                            out=hd_sb[:, t * 2 * H:(t + 1) * 2 * H],
                            in0=ps[:, 2 * H:], in1=bb_sb[li][:], op=ALU.add)
                        nc.sync.dma_start(
                            hsag_in[t * 128:(t + 1) * 128, :], hs_row[:])
                # AllGather hs
                nc.gpsimd.collective_compute(
                    "AllGather", ALU.bypass, replica_groups=RG,
                    ins=[hsag_in[:]], outs=[hs_full[:]])

                # ---- phase A: edges ----
                with (
                    tc.tile_pool(name="pg", bufs=3) as gp,
                    tc.tile_pool(name="pa", bufs=2) as ap,
                    tc.tile_pool(name="pz", bufs=2, space="PSUM") as zp,
                    tc.tile_pool(name="pagg", bufs=2, space="PSUM") as agp,
                    tc.tile_pool(name="pst", bufs=1) as stp,
                ):
                    stats1 = stp.tile([128, NB], F32, tag="st1")
                    stats2 = stp.tile([128, NB], F32, tag="st2")
                    agg_ps = None
                    agg_started = False
                    hcol0 = 0
                    g0 = 0
                    for b in range(NB):
                        ngrp = int(gpb[b])
                        nslots = ngrp * 128
                        # ---- block gathers (bucketed dma_gather) ----
                        hsrows = gp.tile([128, MAXGPB * 2 * H], BF16,
                                         tag="hsrows")
                        scol = 0
                        for k in range(NBK):
                            q = int(quota[b][k])
                            if q == 0:
                                continue
                            lo = k * BK
                            hi = min(NPAD, (k + 1) * BK)
                            nc.gpsimd.dma_gather(
                                out_ap=hsrows[:, scol * 2 * H:(scol + q // 128)
                                              * 2 * H].rearrange(
                                    "p (j c) -> p j c", c=2 * H),
                                in_ap=hs_full[lo:hi, :],
                                idxs_ap=hsix_sb[:, hcol0:hcol0 + q // 16],
                                num_idxs=q, num_idxs_reg=q,
                                elem_size=2 * H, single_packet=True)
                            hcol0 += q // 16
                            scol += q // 128
                        # whole-block ef / S / D streams (one DMA each)
                        ef = ap.tile([128, MAXGPB * 128], BF16, tag="ef")
                        nc.sync.dma_start(
                            ef[:ED, :nslots],
                            efT_d[:, g0 * 128:(g0 + ngrp) * 128])
                        Ssb = ap.tile([128, MAXGPB * 128], BF16, tag="Ssb")
                        nc.sync.dma_start(
                            Ssb[:, :nslots],
                            S_d[:, g0 * 128:(g0 + ngrp) * 128])
                        Dsb = ap.tile([128, MAXGPB * 128], BF16, tag="Dsb")
                        nc.sync.dma_start(
                            Dsb[:, :nslots],
                            D_d[:, g0 * 128:(g0 + ngrp) * 128])
                        # ---- supertiles within block ----
                        for st0 in range(0, ngrp, SUPER):
                            ns = min(SUPER, ngrp - st0)
                            gbase = g0 + st0
                            z = zp.tile([128, SUPER * 2 * H], F32,
                                        space="PSUM", tag="z")
                            for j in range(ns):
                                jj = st0 + j
                                nc.tensor.matmul(
                                    z[:, j * 2 * H:(j + 1) * 2 * H],
                                    lhsT=ef[:ED, jj * 128:(jj + 1) * 128],
                                    rhs=we_sb[li][:ED, :],
                                    start=True, stop=False)
                                nc.tensor.matmul(
                                    z[:, j * 2 * H:(j + 1) * 2 * H],
                                    lhsT=id_bf[:],
                                    rhs=hsrows[:, jj * 2 * H:(jj + 1) * 2 * H],
                                    start=False, stop=False)
                                nc.tensor.matmul(
                                    z[:, j * 2 * H:(j + 1) * 2 * H],
                                    lhsT=Dsb[:, jj * 128:(jj + 1) * 128],
                                    rhs=hd_sb[:, b * 2 * H:(b + 1) * 2 * H],
                                    start=False, stop=True)
                            z3 = z[:, :ns * 2 * H].rearrange(
                                "p (g c) -> p g c", c=2 * H)
                            u = ap.tile([128, SUPER * H], F32, tag="u")
                            u3 = u[:, :ns * H].rearrange(
                                "p (g c) -> p g c", c=H)
                            nc.scalar.activation(u3, z3[:, :, 0:H], AF.Exp)
                            v = ap.tile([128, SUPER * H], F32, tag="v")
                            v3 = v[:, :ns * H].rearrange(
                                "p (g c) -> p g c", c=H)
                            nc.scalar.activation(v3, z3[:, :, H:2 * H], AF.Exp)
                            sp_t = ap.tile([128, SUPER * H], F32, tag="spt")
                            nc.scalar.activation(sp_t[:, :ns * H],
                                                 v[:, :ns * H], AF.Ln,
                                                 bias=1.0)
                            w1 = ap.tile([128, SUPER * H], F32, tag="w1")
                            nc.scalar.activation(w1[:, :ns * H],
                                                 u[:, :ns * H], AF.Copy,
                                                 bias=1.0)
                            r = ap.tile([128, SUPER * H], F32, tag="r")
                            nc.vector.reciprocal_approx_fast(
                                out=r[:, :ns * H], in_=w1[:, :ns * H])
                            msg = ap.tile([128, SUPER * H], BF16, tag="msg")
                            nc.vector.scalar_tensor_tensor(
                                out=msg[:, :ns * H], in0=r[:, :ns * H],
                                scalar=1.0, in1=sp_t[:, :ns * H],
                                op0=ALU.subtract, op1=ALU.mult)

                            for j in range(ns):
                                g = gbase + j
                                jj = st0 + j
                                if agg_ps is None:
                                    agg_ps = agp.tile([128, 128], F32,
                                                      space="PSUM",
                                                      tag="aggps")
                                    agg_started = False
                                nc.tensor.matmul(
                                    agg_ps[:],
                                    lhsT=msg[:, j * H:(j + 1) * H],
                                    rhs=Ssb[:, jj * 128:(jj + 1) * 128],
                                    start=not agg_started,
                                    stop=(g in last_grps))
                                agg_started = True
                                if g in last_grps:
                                    af = ap.tile([128, 128], F32, tag="af")
                                    nc.vector.tensor_copy(af[:], agg_ps[:])
                                    sq = ap.tile([128, 128], F32, tag="sq")
                                    nc.vector.tensor_tensor(
                                        out=sq[:], in0=af[:], in1=af[:],
                                        op=ALU.mult)
                                    nc.vector.tensor_reduce(
                                        stats1[:, b:b + 1], af[:],
                                        axis=mybir.AxisListType.X, op=ALU.add)
                                    nc.vector.tensor_reduce(
                                        stats2[:, b:b + 1], sq[:],
                                        axis=mybir.AxisListType.X, op=ALU.add)
                                    nc.sync.dma_start(
                                        agg_dram[:, b * 128:(b + 1) * 128],
                                        af[:])
                                    agg_ps = None
                        g0 += ngrp

                    # ---- layer stats: sums + AllReduce ----
                    stt = stp.tile([H, 2], F32, tag="stt")
                    nc.vector.tensor_reduce(stt[:, 0:1], stats1[:],
                                            axis=mybir.AxisListType.X,
                                            op=ALU.add)
                    nc.vector.tensor_reduce(stt[:, 1:2], stats2[:],
                                            axis=mybir.AxisListType.X,
                                            op=ALU.add)
                    nc.sync.dma_start(stat_in[:], stt[:])
                nc.gpsimd.collective_compute(
                    "AllReduce", ALU.add, replica_groups=RG,
                    ins=[stat_in[:]], outs=[stat_out[:]])

                if debug and li == 0:
                    nc.sync.dma_start(dbg["dbg_agg0"][:, :], agg_dram[:, :])
                    nc.sync.dma_start(dbg["dbg_st0"][:, :], stat_out[:])
                # ---- phase S: bn scale/shift ----
                with tc.tile_pool(name="ps2", bufs=1) as sp2:
                    stf = sp2.tile([H, 2], F32, tag="stf")
                    nc.sync.dma_start(stf[:], stat_out[:])
                    mv = sp2.tile([H, 4], F32, tag="mv")
                    # mv: 0=mean, 1=ex2, 2=var, 3=scratch
                    nc.vector.tensor_scalar_mul(mv[:, 0:1], stf[:, 0:1], 1.0 / N)
                    nc.vector.tensor_scalar_mul(mv[:, 1:2], stf[:, 1:2], 1.0 / N)
                    nc.vector.tensor_tensor(out=mv[:, 3:4], in0=mv[:, 0:1],
                                            in1=mv[:, 0:1], op=ALU.mult)
                    nc.vector.tensor_tensor(out=mv[:, 2:3], in0=mv[:, 1:2],
                                            in1=mv[:, 3:4], op=ALU.subtract)
                    nc.vector.tensor_scalar_add(mv[:, 2:3], mv[:, 2:3], EPS)
                    lnv = sp2.tile([H, 1], F32, tag="lnv")
                    nc.scalar.activation(lnv[:], mv[:, 2:3], AF.Ln)
                    nc.vector.tensor_scalar_mul(lnv[:], lnv[:], -0.5)
                    rs = sp2.tile([H, 1], F32, tag="rs")
                    nc.scalar.activation(rs[:], lnv[:], AF.Exp)
                    nc.vector.tensor_tensor(out=scsh_sb[:, 0:1], in0=rs[:],
                                            in1=bng_sb[li][:], op=ALU.mult)
                    nc.vector.tensor_tensor(out=mv[:, 3:4], in0=mv[:, 0:1],
                                            in1=scsh_sb[:, 0:1], op=ALU.mult)
                    nc.vector.tensor_tensor(out=scsh_sb[:, 1:2],
                                            in0=bnb_sb[li][:], in1=mv[:, 3:4],
                                            op=ALU.subtract)

                # ---- phase U: h update (+ pooling on last layer) ----
                last = li == NL - 1
                with (
                    tc.tile_pool(name="pu", bufs=4) as up,
                    tc.tile_pool(name="pup", bufs=2, space="PSUM") as upp,
                    tc.tile_pool(name="pug", bufs=1, space="PSUM") as ugp,
                ):
                    if last:
                        g_pool_psum = ugp.tile([B, H], F32, space="PSUM",
                                               tag="gpool")
                    for t in range(NB):
                        ht = up.tile([128, 128], BF16, tag="ht")
                        nc.sync.dma_start(ht[:],
                                          hT_dram[:, t * 128:(t + 1) * 128])
                        agt = up.tile([128, 128], F32, tag="agt")
                        nc.sync.dma_start(agt[:],
                                          agg_dram[:, t * 128:(t + 1) * 128])
                        t1 = up.tile([128, 128], F32, tag="t1")
                        nc.vector.scalar_tensor_tensor(
                            out=t1[:], in0=agt[:], scalar=scsh_sb[:, 0:1],
                            in1=scsh_sb[:, 1:2].to_broadcast([128, 128]),
                            op0=ALU.mult, op1=ALU.add)
                        t2 = up.tile([128, 128], F32, tag="t2")
                        nc.vector.tensor_tensor(out=t2[:], in0=t1[:],
                                                in1=ht[:], op=ALU.add)
                        hnew = up.tile([128, 128], BF16, tag="hnew")
                        nc.vector.tensor_scalar_max(hnew[:], t2[:], 0.0)
                        nc.sync.dma_start(hT_dram[:, t * 128:(t + 1) * 128],
                                          hnew[:])
                        if last:
                            tp = upp.tile([128, 128], BF16, space="PSUM",
                                          tag="tp")
                            nc.tensor.transpose(out=tp[:], in_=hnew[:],
                                                identity=id_bf[:])
                            hbk = up.tile([128, 128], BF16, tag="hbk")
                            nc.vector.tensor_copy(hbk[:], tp[:])
                            Bt = up.tile([128, B], BF16, tag="Bt")
                            nc.sync.dma_start(Bt[:],
                                              Bt_d[:, t * B:(t + 1) * B])
                            nc.tensor.matmul(g_pool_psum[:], lhsT=Bt[:],
                                             rhs=hbk[:], start=(t == 0),
                                             stop=(t == NB - 1))
                    if last:
                        gsb = up.tile([B, H], F32, tag="gsb")
                        nc.vector.tensor_copy(gsb[:], g_pool_psum[:])
                        nc.sync.dma_start(g_in[:], gsb[:])

            # ============ head ============
            nc.gpsimd.collective_compute(
                "AllReduce", ALU.add, replica_groups=RG,
                ins=[g_in[:]], outs=[g_out[:]])
            if debug:
                nc.sync.dma_start(dbg["dbg_g"][:, :], g_out[:])
            with (
                tc.tile_pool(name="hd2", bufs=1) as hp2,
                tc.tile_pool(name="hdp2", bufs=2, space="PSUM") as hpp2,
            ):
                gsum = hp2.tile([B, H], F32, tag="gsum")
                nc.sync.dma_start(gsum[:], g_out[:])
                gmean = hp2.tile([B, H], F32, tag="gmean")
                nc.vector.tensor_scalar_mul(gmean[:], gsum[:], cinv_sb[:, 0:1])
                tps = hpp2.tile([H, B], F32, space="PSUM", tag="tps")
                nc.tensor.transpose(out=tps[:], in_=gmean[:],
                                    identity=id_f32[:B, :B])
                gT = hp2.tile([H, B], F32, tag="gT")
                nc.vector.tensor_copy(gT[:], tps[:])
                x1p = hpp2.tile([H, B], F32, space="PSUM", tag="x1p")
                nc.tensor.matmul(x1p[:], lhsT=fc1w_sb[:], rhs=gT[:],
                                 start=True, stop=True)
                x1 = hp2.tile([H, B], F32, tag="x1")
                nc.vector.scalar_tensor_tensor(
                    out=x1[:], in0=x1p[:], scalar=1.0,
                    in1=fc1b_sb[:].to_broadcast([H, B]),
                    op0=ALU.mult, op1=ALU.add)
                # bn over graphs (free dim)
                sc2 = hp2.tile([H, 6], F32, tag="sc2")
                # cols: 0 sum,1 mean,2 ex2,3 var/misc,4 sc,5 sh
                nc.vector.tensor_reduce(sc2[:, 0:1], x1[:],
                                        axis=mybir.AxisListType.X, op=ALU.add)
                nc.vector.tensor_scalar_mul(sc2[:, 1:2], sc2[:, 0:1], 1.0 / B)
                xsq = hp2.tile([H, B], F32, tag="xsq")
                nc.vector.tensor_tensor(out=xsq[:], in0=x1[:], in1=x1[:],
                                        op=ALU.mult)
                nc.vector.tensor_reduce(sc2[:, 2:3], xsq[:],
                                        axis=mybir.AxisListType.X, op=ALU.add)
                nc.vector.tensor_scalar_mul(sc2[:, 2:3], sc2[:, 2:3], 1.0 / B)
                nc.vector.tensor_tensor(out=sc2[:, 3:4], in0=sc2[:, 1:2],
                                        in1=sc2[:, 1:2], op=ALU.mult)
                nc.vector.tensor_tensor(out=sc2[:, 3:4], in0=sc2[:, 2:3],
                                        in1=sc2[:, 3:4], op=ALU.subtract)
                nc.vector.tensor_scalar_add(sc2[:, 3:4], sc2[:, 3:4], EPS)
                lnv2 = hp2.tile([H, 1], F32, tag="lnv2")
                nc.scalar.activation(lnv2[:], sc2[:, 3:4], AF.Ln)
                nc.vector.tensor_scalar_mul(lnv2[:], lnv2[:], -0.5)
                rs2 = hp2.tile([H, 1], F32, tag="rs2")
                nc.scalar.activation(rs2[:], lnv2[:], AF.Exp)
                nc.vector.tensor_tensor(out=sc2[:, 4:5], in0=rs2[:],
                                        in1=fcg_sb[:], op=ALU.mult)
                nc.vector.tensor_tensor(out=sc2[:, 3:4], in0=sc2[:, 1:2],
                                        in1=sc2[:, 4:5], op=ALU.mult)
                nc.vector.tensor_tensor(out=sc2[:, 5:6], in0=fcb_sb[:],
                                        in1=sc2[:, 3:4], op=ALU.subtract)
                x2 = hp2.tile([H, B], F32, tag="x2")
                nc.vector.scalar_tensor_tensor(
                    out=x2[:], in0=x1[:], scalar=sc2[:, 4:5],
                    in1=sc2[:, 5:6].to_broadcast([H, B]),
                    op0=ALU.mult, op1=ALU.add)
                x2r = hp2.tile([H, B], F32, tag="x2r")
                nc.vector.tensor_scalar_max(x2r[:], x2[:], 0.0)
                yp = hpp2.tile([1, B], F32, space="PSUM", tag="yp")
                nc.tensor.matmul(yp[:], lhsT=outw_sb[:], rhs=x2r[:],
                                 start=True, stop=True)
                ysb = hp2.tile([1, B], F32, tag="ysb")
                nc.vector.tensor_scalar_add(ysb[:], yp[:], meta["out_b"])
                nc.sync.dma_start(out_d[:].rearrange("b o -> o b"), ysb[:])

    return nc


# --------------------------------------------------------------------------
def run(inputs, cfg=None, debug=False):
    cfg = cfg or DEFAULT_CFG
    meta, in_maps = _preprocess(cfg, inputs)
    nc = _build(cfg, meta, debug=debug)
    nc.finalize()
    res = run_bass_kernel_spmd(nc, in_maps, core_ids=list(range(NCORES)))
    if debug:
        return res.results
    return np.asarray(res.results[0]["out"], np.float32)


def kernel(**inputs):
    return run(inputs, DEFAULT_CFG)

